# revision 1
# baseline (speedup 1.0000x reference)
"""Trainium2 Bass kernel for the AF3-style diffusion loss.

Contract: kernel(**inputs) takes the FULL inputs (as in reference.setup_inputs)
and returns the FULL scalar output.

Strategy (8 NeuronCores):
  - Data-parallel over batch (B=2) x 4 row-blocks of 512 atoms -> 8 shards.
  - Each core computes, for its 512x2048 slice of the pairwise matrices:
      s15[l]  = sum_j (d_gt < 15)
      s30[l]  = sum_j (d_gt < 30)
      s15e[l] = sum_j (d_gt < 15) * e4      (e4 = sum of 4 sigmoids, unscaled)
      s30e[l] = sum_j (d_gt < 30) * e4
      bond[l] = sum_t (sum_{j in token t} (dx-dgt)^2) * w_bond[l,t]
    Distances come from a K=5 fp32 matmul (d^2 = |xi|^2+|xj|^2-2 xi.xj) into
    PSUM, then max(.,eps) + sqrt on ACT.
  - Host (numpy, O(N) / O(T^2) only): token one-hot features, bond weights,
    denominators, diagonal corrections, the 3x3 Kabsch solve + weighted MSE,
    and the final combine.
"""

import os
import numpy as np

B, A, T, APT = 2, 2048, 256, 8
NCORES = 8
RB = A // 4          # 512 rows per core
NT = RB // 128       # 4 row tiles per core
CH = 512             # col chunk (max matmul moving free dim)
NCH = A // CH        # 4 col chunks
NV = 5               # accum values per (tile, chunk)
GW = 8 + 512         # output cols per row-tile group (4 sums x 2 + 256*2 blk)
OUTW = 2 * GW        # device output width
SIGMA_DATA = 16.0
E0 = 0.25 * sum(1.0 / (1.0 + np.exp(-z)) for z in (0.5, 1.0, 2.0, 4.0))

_CACHE = {}
LAST_RESULTS = None  # test.py reads exec_time_ns from here


def _build_bass(reps=1):
    """Matmul-free build: pairwise d^2 via wide DVE ops with 0-stride
    broadcast access patterns (row coords broadcast along free, column
    coords replicated across partitions by DMA), one in-place Sqrt, then
    the threshold/sigmoid phase. ~53 instructions per rep; PE unused.
    SBUF tiles are role-aliased (scratch: diff/sq -> sigmoid quad;
    xcb: column coords -> threshold pack + d2) to fit the budget."""
    import concourse.bacc as bacc
    import concourse.mybir as mybir
    from concourse.tile import TileContext

    f32 = mybir.dt.float32
    Alu = mybir.AluOpType
    AF = mybir.ActivationFunctionType

    nc = bacc.Bacc(None, target_bir_lowering=False)
    MMW = 2 * RB + 2 * A
    mm_d = nc.dram_tensor("mm", [5, MMW], f32, kind="ExternalInput")
    xr_d = nc.dram_tensor("xr", [128, 24], f32, kind="ExternalInput")
    out_d = nc.dram_tensor("out", [128, OUTW], f32, kind="ExternalOutput")

    with TileContext(nc) as tc:
        with (
            tc.tile_pool(name="cpool", bufs=1) as cp,
            tc.tile_pool(name="wpool", bufs=1) as wp,
        ):
            xr_st = cp.tile([128, 24], f32, name="xr_st", tag="xr_st")
            xr = cp.tile([128, 24], f32, name="xr_sb", tag="xr_sb")
            outb = cp.tile([128, OUTW], f32, name="out_sb", tag="out_sb")
            nc.sync.dma_start(xr_st[:], xr_d[:])
            nc.vector.tensor_copy(xr[:], xr_st[:])
            xrv = xr[:].rearrange("p (s d t) -> p s d t", s=2, d=3)

            def act_const(val, nm):
                st = cp.tile([128, 1], f32, name=nm + "_st", tag=nm + "_st")
                nc.vector.memset(st[:], val)
                fin = cp.tile([128, 1], f32, name=nm, tag=nm)
                nc.scalar.activation(fin[:], st[:], AF.Copy)
                return fin
            bias0 = act_const(0.0, "bias0")
            biaseps = act_const(1e-12, "biaseps")
            btau = [act_const(float(tau), f"btau{k}")
                    for k, tau in enumerate((0.5, 1.0, 2.0, 4.0))]

            da = cp.tile([128, 16384], f32, name="da", tag="da")
            scratch = cp.tile([128, 8192], f32, name="scratch", tag="scratch")
            xcb = cp.tile([128, 12288], f32, name="xcb", tag="xcb")
            sd = cp.tile([128, 8192], f32, name="sd", tag="sd")

            for rep in range(reps):
                # column coords, one partition-broadcast DMA from mm DRAM:
                # block (s,d) at xcb[:, (s*3+d)*A :]
                for sdi in range(6):
                    si0, di0 = divmod(sdi, 3)
                    srcb = mm_d[di0:di0 + 1,
                                2 * RB + si0 * A:2 * RB + (si0 + 1) * A]
                    nc.sync.dma_start(xcb[:, sdi * A:(sdi + 1) * A],
                                      srcb.broadcast_to((128, A)))
                dav = da[:].rearrange("p (t s c) -> p t s c", t=4, s=2)
                ds = scratch[:].rearrange("p (t c) -> p t c", t=4)
                tmp = sd[:].rearrange("p (t c) -> p t c", t=4)
                for si in range(2):
                    tgt = dav[:, :, si, :]
                    for di in range(3):
                        xc = xcb[:, (si * 3 + di) * A:(si * 3 + di + 1) * A]
                        xcv = xc.unsqueeze(1).broadcast_to((128, 4, A))
                        rv = xrv[:, si, di, :].unsqueeze(2).broadcast_to((128, 4, A))
                        nc.vector.tensor_sub(ds, xcv, rv)
                        if di == 0:
                            nc.vector.tensor_mul(tgt, ds, ds)
                        else:
                            nc.vector.tensor_mul(tmp, ds, ds)
                            nc.vector.tensor_add(tgt, tgt, tmp)
                nc.scalar.activation(da[:], da[:], AF.Sqrt, bias=biaseps[:])

                # full-wide downstream (no group loop)
                dxv = dav[:, :, 0, :]
                dgv = dav[:, :, 1, :]
                sdv = sd[:].rearrange("p (t c) -> p t c", t=4)
                nc.vector.tensor_sub(sdv, dgv, dxv)
                d2 = xcb[:, 0:8192]
                nc.vector.tensor_mul(d2, sd[:], sd[:])
                d2q = d2.rearrange("p (t k e) -> p t k e", t=4, e=APT)
                blko = outb[:, 16:16 + 1024].rearrange("p (t k) -> p t k", t=4)
                nc.vector.tensor_reduce(blko, d2q,
                                        axis=mybir.AxisListType.X, op=Alu.add)
                nc.scalar.activation(sd[:], sd[:], AF.Abs, bias=bias0[:])
                ea = scratch[:]
                eb = xcb[:, 0:8192]
                nc.scalar.activation(ea, sd[:], AF.Sigmoid,
                                     bias=btau[0][:], scale=-1.0)
                for k in (1, 2, 3):
                    nc.scalar.activation(eb, sd[:], AF.Sigmoid,
                                         bias=btau[k][:], scale=-1.0)
                    nc.vector.tensor_add(ea, ea, eb)
                # thresholds: q15 reuses sd (|d| dead), q30 reuses eb
                q15 = sd[:].rearrange("p (t c) -> p t c", t=4)
                q30 = eb.rearrange("p (t c) -> p t c", t=4)
                nc.vector.tensor_scalar(q15, dgv, 15.0, None, Alu.is_lt)
                nc.vector.tensor_scalar(q30, dgv, 30.0, None, Alu.is_lt)
                nc.vector.tensor_reduce(outb[:, 0:4], q15,
                                        axis=mybir.AxisListType.X, op=Alu.add)
                nc.vector.tensor_reduce(outb[:, 4:8], q30,
                                        axis=mybir.AxisListType.X, op=Alu.add)
                nc.vector.tensor_mul(sd[:], sd[:], ea)
                nc.vector.tensor_mul(eb, eb, ea)
                nc.vector.tensor_reduce(outb[:, 8:12], q15,
                                        axis=mybir.AxisListType.X, op=Alu.add)
                nc.vector.tensor_reduce(outb[:, 12:16], q30,
                                        axis=mybir.AxisListType.X, op=Alu.add)

            nc.sync.dma_start(out_d[:], outb[:])
    nc.compile()
    return nc


def _tok_features(isp, isd, isr, isl, tb, tm, npt):
    """Token->atom features, general in npt/tm. All numpy, O(A*T)."""
    cum = np.cumsum(npt, -1)
    start = cum - npt
    l = np.arange(A)
    ind = ((l[:, None] >= start[:, None, :]) & (l[:, None] < cum[:, None, :]))
    ind = ind.astype(np.float32)                      # [B,A,T] pure indicator
    oh = ind * tm[:, None, :]
    is_nuc = np.einsum('blt,bt->bl', oh, isd + isr)
    w_tok = 1.0 + isd * 5.0 + isr * 5.0 + isl * 10.0
    w_atom = np.einsum('blt,bt->bl', oh, w_tok)
    is_poly = isp + isd + isr
    tbm = tb * (is_poly[:, None, :] * isl[:, :, None]) * tm[:, None, :] * tm[:, :, None]
    wb_full = np.einsum('blt,btj->blj', ind, tbm)     # [B,A,T] bond row weights
    return oh, ind, is_nuc, w_atom, tbm, wb_full


def _mse_host(x, gt, gm, w_atom):
    """Weighted rigid align (Kabsch) of gt onto x + weighted MSE. Per sample."""
    denom = gm.sum()
    w_mean = (w_atom * gm).sum() / denom
    wm = (w_atom * gm)[:, None]
    mu = (gt * wm).sum(0) / denom / w_mean
    mu_gt = (x * wm).sum(0) / denom / w_mean
    xc = gt - mu
    xgc = x - mu_gt
    H = (xgc * wm).T @ xc
    U, _, Vh = np.linalg.svd(H)
    det = np.linalg.det(U @ Vh)
    s = np.array([1.0, 1.0, np.sign(det)])
    R = U @ (Vh * s[:, None])
    gt_al = xc @ R.T + mu_gt
    return (1.0 / 3.0) * (((x - gt_al) ** 2).sum(-1) * w_atom * gm).sum() / denom


def _numpy_fallback(x, gt, gm, isp, isd, isr, isl, tb, tm, npt, t):
    """Full-precision numpy port of the reference; used only when the inputs
    fall outside the fast-path assumptions (non-uniform atoms/masks)."""
    oh, ind, is_nuc, w_atom, tbm, wb_full = _tok_features(isp, isd, isr, isl, tb, tm, npt)
    sig = lambda z: 1.0 / (1.0 + np.exp(-z))
    loss = 0.0
    for b in range(B):
        d = x[b][:, None, :] - x[b][None, :, :]
        dx = np.sqrt((d * d).sum(-1) + 1e-12)
        d = gt[b][:, None, :] - gt[b][None, :, :]
        dg = np.sqrt((d * d).sum(-1) + 1e-12)
        pm = gm[b][:, None] * gm[b][None, :]
        bm = ind[b] @ tbm[b] @ ind[b].T
        m = bm * pm
        lb = (((dx - dg) ** 2) * m).sum() / m.sum()
        dd = np.abs(dg - dx)
        e = 0.25 * (sig(0.5 - dd) + sig(1.0 - dd) + sig(2.0 - dd) + sig(4.0 - dd))
        c = (dg < 30) * is_nuc[b][:, None] + (dg < 15) * (1.0 - is_nuc[b][:, None])
        m2 = (1.0 - np.eye(A)) * pm
        msum = m2.sum()
        ll = 1.0 - ((c * e * m2).sum() / msum) / ((c * m2).sum() / msum)
        lm = _mse_host(x[b], gt[b], gm[b], w_atom[b])
        wt = (t[b] ** 2 + SIGMA_DATA ** 2) / (t[b] + SIGMA_DATA) ** 2
        loss += wt * (lm + lb) + ll
    return np.float32(loss / B)


def kernel(x, gt_atom_positions, gt_atom_mask, is_protein, is_dna, is_rna,
           is_ligand, token_bonds, token_mask, num_atoms_per_token, t):
    global LAST_RESULTS
    f = np.asarray
    x = f(x, np.float32)
    gt = f(gt_atom_positions, np.float32)
    gm = f(gt_atom_mask, np.float32)
    isp, isd, isr, isl = (f(v, np.float32) for v in
                          (is_protein, is_dna, is_rna, is_ligand))
    tb = f(token_bonds, np.float32)
    tm = f(token_mask, np.float32)
    npt = f(num_atoms_per_token, np.int32)
    t = f(t, np.float32)

    fast = bool(np.all(npt == APT)) and bool(np.all(gm == 1.0))
    if not fast:
        return _numpy_fallback(x, gt, gm, isp, isd, isr, isl, tb, tm, npt, t)

    oh, ind, is_nuc, w_atom, tbm, wb_full = _tok_features(isp, isd, isr, isl, tb, tm, npt)

    # Per-core device inputs: core c -> sample b=c//4, rows [512r, 512r+512)
    in_maps = []
    for c in range(NCORES):
        b, r = divmod(c, 4)
        rows = slice(RB * r, RB * (r + 1))
        xb, gb = x[b], gt[b]
        ni = (xb * xb).sum(-1)
        gi = (gb * gb).sum(-1)
        ones = np.ones(A, np.float32)

        def packs(coords, nrm, sl):
            m = np.empty((5, RB), np.float32)
            m[0:3] = -2.0 * coords[sl].T
            m[3] = nrm[sl] + 1e-3   # keeps d^2 > 0 (see device Sqrt eviction)
            m[4] = 1.0
            return m

        def packr(coords, nrm):
            m = np.empty((5, A), np.float32)
            m[0:3] = coords.T
            m[3] = 1.0
            m[4] = nrm
            return m

        mm = np.empty((5, 2 * RB + 2 * A), np.float32)
        mm[:, 0:RB] = packs(xb, ni, rows)
        mm[:, RB:2 * RB] = packs(gb, gi, rows)
        mm[:, 2 * RB:2 * RB + A] = packr(xb, ni)
        mm[:, 2 * RB + A:2 * RB + 2 * A] = packr(gb, gi)
        # xr[p, (s,d,t)] = coords_s[512r + 128t + p, d]
        xrh = np.empty((128, 2, 3, 4), np.float32)
        for si, coords in enumerate((xb, gb)):
            blkc = coords[rows].reshape(4, 128, 3)     # [t, p, d]
            xrh[:, si] = blkc.transpose(1, 2, 0)       # [p, d, t]
        in_maps.append({"mm": mm, "xr": xrh.reshape(128, 24)})

    if "nc" not in _CACHE:
        _CACHE["nc"] = _build_bass()
    os.environ.setdefault("BASS_NEVER_TRACE", "1")
    from concourse.bass_utils import run_bass_kernel_spmd
    res = run_bass_kernel_spmd(_CACHE["nc"], in_maps, core_ids=list(range(NCORES)))
    LAST_RESULTS = res
    globals()["LAST_IN_MAPS"] = in_maps

    # Host combine. Device layout per group g: cols g*GW + [s15(2) s30(2)
    # s15e(2) s30e(2) blk(2x256)], segment t inside = row-tile 2g+t.
    loss = 0.0
    for b in range(B):
        s15 = np.empty(A, np.float64); s30 = np.empty(A, np.float64)
        s15e = np.empty(A, np.float64); s30e = np.empty(A, np.float64)
        blk = np.empty((A, T), np.float64)
        for r in range(4):
            o = res.results[4 * b + r]["out"]  # [128, OUTW]
            for seg in range(4):
                base = RB * r + 128 * seg
                s15[base:base + 128] = o[:, seg]
                s30[base:base + 128] = o[:, 4 + seg]
                s15e[base:base + 128] = o[:, 8 + seg]
                s30e[base:base + 128] = o[:, 12 + seg]
                blk[base:base + 128] = o[:, 16 + seg * 256:16 + (seg + 1) * 256]
        bond = (blk * wb_full[b]).sum(-1)
        nuc = is_nuc[b].astype(np.float64)
        c_rows = s15 + nuc * (s30 - s15) - 1.0
        ce_rows = 0.25 * (s15e + nuc * (s30e - s15e)) - E0
        ll = 1.0 - ce_rows.sum() / c_rows.sum()
        a_i = ind[b].T @ gm[b].astype(np.float32)     # atoms per token (masked)
        bond_den = float(a_i @ tbm[b] @ a_i)
        lb = bond.sum() / bond_den
        lm = _mse_host(x[b], gt[b], gm[b], w_atom[b])
        wt = (t[b] ** 2 + SIGMA_DATA ** 2) / (t[b] + SIGMA_DATA) ** 2
        loss += wt * (lm + lb) + ll
    return np.float32(loss / B)

